# revision 1
# baseline (speedup 1.0000x reference)
"""Trilerp kernel v3: bulk dma_gather replaces per-column indirect DMAs.

Host groups same-cell points (k in 4..1), sorts groups by class (cell&7),
and ships precomputed 8-corner weights. Device: per chunk, a few dma_gather
instructions (64B rows from a 512B-stride R64 view, class-sliced base) fill
g[P,G,16]; blend = mult+reduce per member column-group. ~265 gather
instructions/core of 1024 idxs each (64 descs/engine = HW packet ceiling;
994ns SWDGE fixed each) instead of 2048 indirect DMAs of 128 descs.
"""
import sys
sys.path.insert(0, '/opt/trn_rl_repo')
import numpy as np

import concourse.bass as bass
import concourse.mybir as mybir
from concourse import bacc
from concourse import ap_utils
from concourse.tile import TileContext
from concourse.bass_utils import run_bass_kernel_spmd
from concourse.library_config import mlp

RES = 128
F = 2
NCORES = 8
P = 128
G = 192            # gather columns per chunk
PIECE = 8          # max columns per dma_gather (1024 idxs, Q7 scratch limit)
ROWS = 16 * RES * RES
NBLK = ROWS // 8   # 32768 512B blocks
KS = (4, 3, 2, 1)
_F32 = mybir.dt.float32
_I16 = mybir.dt.int16
_LAST = {}


def emit_dma_gather(gp, out_ap, in_ap, idxs_ap, num_idxs, elem_size, elem_step):
    """bass.BassGpSimd.dma_gather minus the elem_size%256 assert."""
    assert idxs_ap.dtype == mybir.dt.int16
    assert ap_utils.ap_is_contiguous(out_ap.ap[1:])
    assert ap_utils.ap_is_contiguous(idxs_ap.ap[1:])
    assert in_ap.ap[-1][1] == out_ap.ap[-1][1] == elem_size
    assert in_ap.ap[0][0] == elem_step
    stride_bytes = elem_step * mybir.dt.size(in_ap.dtype)
    assert stride_bytes % 256 == 0
    _in_ap = gp.lower_ap_dma(in_ap, for_custom_bir_dma=True)
    _idxs_ap = gp.lower_ap(idxs_ap)
    _out_ap = gp.lower_ap(out_ap)
    return gp.add_instruction(
        mybir.InstDMAGatherAnt(
            name=gp.bass.get_next_instruction_name(),
            ins=[*_in_ap, _idxs_ap, gp.lower_val_access(gp.to_reg(num_idxs))],
            outs=[_out_ap],
            transpose=False, num_idxs=num_idxs, elem_size=elem_size,
            stride_bytes_256=stride_bytes // 256, gen_mode=0,
            single_packet=True, queue_num=0, sbuf_tokens_per_rank=0,
            sbuf_free_dim_per_rank=0, sbuf_free_dim_pad_per_rank=0,
            sbuf_byte_offset=0,
        ))


def build_core_kernel(layout):
    """layout: dict k -> (n_chunks, pieces) where pieces is a list per chunk of
    (cls, ga, gb) column ranges; plus layout['wtot'], layout['slots'] totals."""
    wtot = layout["wtot"]
    tot_slots = layout["tot_slots"]
    nc = bacc.Bacc("TRN2", target_bir_lowering=False, debug=False,
                   num_devices=NCORES, num_swdge_queues=4)
    R8 = nc.dram_tensor("R8", [NBLK, 8 * 16], _F32, kind="ExternalInput")
    idx_d = nc.dram_tensor("idx", [32, tot_slots // 16], _I16, kind="ExternalInput")
    w8_d = nc.dram_tensor("w8", [P, wtot, 8], _F32, kind="ExternalInput")
    out = nc.dram_tensor("out", [P, wtot, F], _F32, kind="ExternalOutput")
    with TileContext(nc) as tc:
        with tc.tile_pool(name="io", bufs=1) as io, \
             tc.tile_pool(name="wk", bufs=2) as wk, \
             tc.tile_pool(name="gp", bufs=2) as gpool:
            nc.gpsimd.load_library(mlp)
            idx_sb = io.tile([32, tot_slots // 16], _I16)
            nc.sync.dma_start(out=idx_sb[:], in_=idx_d[:])
            base_w = 0
            base_s = 0
            for k in KS:
                n_chunks, pieces_by_chunk = layout[k]
                for q in range(n_chunks):
                    w = k * G
                    col0 = base_w + q * w
                    slot0 = base_s + q * G * P
                    g = gpool.tile([P, G, 16], _F32, tag="g")
                    for (cls, ga, gb) in pieces_by_chunk[q]:
                        ni = (gb - ga) * P
                        iofs = (slot0 + ga * P) // 16
                        emit_dma_gather(
                            nc.gpsimd, g[:, ga:gb, :],
                            R8[:, 16 * cls:16 * cls + 16],
                            idx_sb[:, iofs:iofs + ni // 16], ni, 16, 128)
                    w8t = wk.tile([P, w, 8], _F32, tag="w8")
                    nc.sync.dma_start(out=w8t[:], in_=w8_d[:, col0:col0 + w, :])
                    oc = wk.tile([P, w, F], _F32, tag="oc")
                    gv = g[:].rearrange("p t (a f) -> p t a f", a=8, f=F)
                    for j in range(k):
                        p8 = wk.tile([P, G, 8, F], _F32, tag="p8")
                        nc.vector.tensor_tensor(
                            p8[:], gv,
                            w8t[:, j * G:(j + 1) * G, :].unsqueeze(-1)
                                .broadcast_to([P, G, 8, F]),
                            mybir.AluOpType.mult)
                        nc.vector.tensor_reduce(
                            oc[:, j * G:(j + 1) * G],
                            p8[:].transpose([0, 1, 3, 2]),
                            axis=mybir.AxisListType.X, op=mybir.AluOpType.add)
                    nc.sync.dma_start(out=out[:, col0:col0 + w, :], in_=oc[:])
                base_w += n_chunks * k * G
                base_s += n_chunks * G * P
    nc.compile()
    return nc


def _build_r64(table, x0):
    T = np.ascontiguousarray(table, dtype=np.float32)
    xi = np.minimum(x0 + np.arange(16), RES - 1)
    out = np.empty((16, RES, RES, 4, 2, F), np.float32)
    k0 = np.arange(RES)
    k1 = np.minimum(k0 + 1, RES - 1)
    for dx in (0, 1):
        xs = np.minimum(xi + dx, RES - 1)
        for dy in (0, 1):
            ys = np.minimum(np.arange(RES) + dy, RES - 1)
            A = T[xs][:, ys]
            out[:, :, :, dx * 2 + dy, 0, :] = A[:, :, k0, :]
            out[:, :, :, dx * 2 + dy, 1, :] = A[:, :, k1, :]
    return out.reshape(NBLK, 8 * 16)


def kernel(c0, c1, c2, table):
    c0 = np.asarray(c0, np.float32)
    c1 = np.asarray(c1, np.float32)
    c2 = np.asarray(c2, np.float32)
    table = np.asarray(table, np.float32)
    N = c0.shape[0]

    xs = [a * np.float32(RES - 1) for a in (c0, c1, c2)]
    i0 = [np.clip(np.floor(x).astype(np.int64), 0, RES - 2) for x in xs]
    fr = [x - i for x, i in zip(xs, i0)]
    # 8 corner weights, order a = (dx*2+dy)*2 + kz
    W8 = np.empty((N, 8), np.float32)
    for dx in (0, 1):
        wx = fr[0] if dx else 1.0 - fr[0]
        for dy in (0, 1):
            wy = fr[1] if dy else 1.0 - fr[1]
            for kz in (0, 1):
                wz = fr[2] if kz else 1.0 - fr[2]
                W8[:, (dx * 2 + dy) * 2 + kz] = wx * wy * wz
    buckets = i0[0] >> 4
    m_all = (i0[0] - 16 * buckets) * 16384 + i0[1] * 128 + i0[2]

    # per-core grouping: groups[(c,k)] = (gcells, [member pt arrays j=0..k-1])
    per_core = {}
    ccounts = np.zeros((NCORES, len(KS), 8), np.int64)
    for c in range(NCORES):
        idx_c = np.flatnonzero(buckets == c)
        ms = m_all[idx_c]
        order = np.argsort(ms, kind="stable")
        srt = idx_c[order]
        msr = ms[order]
        n = len(srt)
        new_run = np.ones(n, bool)
        if n > 1:
            new_run[1:] = msr[1:] != msr[:-1]
        starts = np.flatnonzero(new_run)
        runlen = np.diff(np.append(starts, n))
        rid = np.cumsum(new_run) - 1
        pos = np.arange(n) - starts[rid]
        rl = runlen[rid]
        nfull = 4 * (rl // 4)
        in_quad = pos < nfull
        k_of = np.where(in_quad, 4, rl % 4)
        j_of = np.where(in_quad, pos % 4, pos - nfull)
        for ki, k in enumerate(KS):
            sel0 = (k_of == k) & (j_of == 0)
            gcells = msr[sel0]
            cls = (gcells & 7).astype(np.int64)
            corder = np.argsort(cls, kind="stable")
            gcells = gcells[corder]
            members = []
            for j in range(k):
                pj = srt[(k_of == k) & (j_of == j)]
                members.append(pj[corder])
            per_core[(c, k)] = (gcells, members)
            np.add.at(ccounts[c, ki], cls, 1)

    # shared layout: per (k, class) padded to 128, max over cores
    layout = {}
    tot_slots = 0
    wtot = 0
    cls_off = {}
    for ki, k in enumerate(KS):
        L = np.max(ccounts[:, ki, :], axis=0)
        L = ((L + P - 1) // P) * P
        offs = np.concatenate([[0], np.cumsum(L)])
        Sk = int(offs[-1])
        n_chunks = max(1, -(-Sk // (G * P)))
        Sk_pad = n_chunks * G * P
        cls_off[k] = offs
        # pieces: class runs cut at chunk boundaries and into <=PIECE cols
        pieces_by_chunk = [[] for _ in range(n_chunks)]
        for q in range(8):
            a, b = int(offs[q]) // P, int(offs[q + 1]) // P  # in columns
            g0 = a
            while g0 < b:
                chunk = g0 // G
                lim = min(b, (chunk + 1) * G, g0 + PIECE)
                pieces_by_chunk[chunk].append((q, g0 - chunk * G, lim - chunk * G))
                g0 = lim
        layout[k] = (n_chunks, pieces_by_chunk)
        tot_slots += Sk_pad
        wtot += n_chunks * k * G
    layout["wtot"] = wtot
    layout["tot_slots"] = tot_slots

    nc = build_core_kernel(layout)
    _LAST["nc"] = nc

    in_maps = []
    slotmaps = []
    for c in range(NCORES):
        IDX = np.zeros(tot_slots, np.int16)
        W8D = np.zeros((P, wtot, 8), np.float32)
        base_s = 0
        base_w = 0
        smap = {}
        for ki, k in enumerate(KS):
            n_chunks, _ = layout[k]
            offs = cls_off[k]
            gcells, members = per_core[(c, k)]
            cls = (gcells & 7).astype(np.int64)
            cnt = ccounts[c, ki]
            first = np.concatenate([[0], np.cumsum(cnt)])[:-1]
            rank = np.arange(len(gcells)) - first[cls]
            slot = offs[cls] + rank            # slot within k-type
            IDX[base_s + slot] = (gcells >> 3).astype(np.int16)
            pcol = slot % P
            gcol = slot // P
            chunk = gcol // G
            gl = gcol % G
            for j in range(k):
                col = base_w + chunk * k * G + j * G + gl
                W8D[pcol, col, :] = W8[members[j]]
            smap[k] = (slot, members, base_w)
            base_s += n_chunks * G * P
            base_w += n_chunks * k * G
        wrapped = IDX.reshape(-1, 16).T
        in_maps.append({
            "R8": _build_r64(table, 16 * c),
            "idx": np.concatenate([wrapped, wrapped], axis=0).copy(),
            "w8": W8D,
        })
        slotmaps.append(smap)

    _LAST["in_maps"] = in_maps
    res = run_bass_kernel_spmd(nc, in_maps, core_ids=list(range(NCORES)))

    out_full = np.empty((N, F), np.float32)
    for c in range(NCORES):
        oc = np.asarray(res.results[c]["out"])
        for k in KS:
            slot, members, base_w = slotmaps[c][k]
            pcol = slot % P
            gcol = slot // P
            chunk = gcol // G
            gl = gcol % G
            for j in range(k):
                col = base_w + chunk * k * G + j * G + gl
                out_full[members[j]] = oc[pcol, col, :]
    return out_full



# revision 3
# speedup vs baseline: 3.2275x; 3.2275x over previous
"""Trilerp kernel v4: ap_gather expansion + PE corner-reduce.

Per core (x-slab of 16): table packed bf16-pair-per-u32 into lanes
[128 = 8 z-class x 2 half x 8 corner], free = cell-pair unit. Host pairs
same-class cells by count (near-zero padding) and ships per-point corner
weights in lane layout. Device: ap_gather (Pool, one free-elem per point,
no DMA descriptors) -> DVE broadcast-mult -> PE matmul corner-reduce with
3 row-block stationaries accumulating into one PSUM bank -> Act drain to
bf16 -> DMA out. No SWDGE gathers at all.
"""
import sys
sys.path.insert(0, '/opt/trn_rl_repo')
import numpy as np
import ml_dtypes

import concourse.bass as bass
import concourse.mybir as mybir
from concourse import bacc
from concourse.tile import TileContext
from concourse.bass_utils import run_bass_kernel_spmd
from concourse.library_config import ap_gather as ap_gather_lib

BF16 = mybir.dt.bfloat16
U32 = mybir.dt.uint32
F32 = mybir.dt.float32
I16 = mybir.dt.int16

RES = 128
NCORES = 8
XL = 16               # x-values per core
NGC = 2048            # cells per (core, x_loc, z-class)
NUNITS = 1024         # cell pairs per (core, x_loc, z-class)
_LAST = {}


def _pack_table(table):
    """[128,128,128,2] f32 -> corner-packed u32 [x,y,z,8a] (bf16 f0|f1<<16)."""
    Tb = table.astype(ml_dtypes.bfloat16).view(np.uint16).astype(np.uint32)
    packed = np.empty((RES, RES, RES, 8), np.uint32)
    ip = np.minimum(np.arange(RES) + 1, RES - 1)
    for a in range(8):
        dx, dy, dz = (a >> 2) & 1, (a >> 1) & 1, a & 1
        V = Tb[ip if dx else slice(None)]
        V = V[:, ip if dy else slice(None)]
        V = V[:, :, ip if dz else slice(None)]
        packed[:, :, :, a] = V[..., 0] | (V[..., 1] << 16)
    return packed


def build_kernel(Ncol, SPX, CPX):
    nc = bacc.Bacc("TRN2", target_bir_lowering=False, debug=False,
                   num_devices=NCORES)
    data_d = nc.dram_tensor("data", [128, XL * NUNITS], U32, kind="ExternalInput")
    idx_d = nc.dram_tensor("idx", [128, XL * Ncol // 16], I16, kind="ExternalInput")
    w_d = nc.dram_tensor("w", [128, XL * Ncol], BF16, kind="ExternalInput")
    s_d = nc.dram_tensor("s", [128, 144], BF16, kind="ExternalInput")
    out_d = nc.dram_tensor("out", [48 * XL, 512 * SPX], BF16, kind="ExternalOutput")
    with TileContext(nc) as tc:
        with tc.tile_pool(name="io", bufs=1) as io, \
             tc.tile_pool(name="tbl", bufs=3) as tbl, \
             tc.tile_pool(name="wk", bufs=2) as wk, \
             tc.tile_pool(name="gp", bufs=2) as gp, \
             tc.tile_pool(name="pp", bufs=2) as pp, \
             tc.tile_pool(name="op", bufs=2) as op, \
             tc.psum_pool(name="ps", bufs=2) as psp:
            nc.gpsimd.load_library(ap_gather_lib)
            s_sb = io.tile([128, 144], BF16, tag="s")
            nc.sync.dma_start(out=s_sb[:], in_=s_d[:])
            idx_sb = io.tile([128, XL * Ncol // 16], I16, tag="idx")
            nc.sync.dma_start(out=idx_sb[:], in_=idx_d[:])
            NI16 = Ncol // 16
            for xl in range(XL):
                if xl % 2 == 0:
                    tb = tbl.tile([128, 2 * NUNITS], U32, tag="tb")
                    nc.sync.dma_start(
                        out=tb[:],
                        in_=data_d[:, xl * NUNITS:(xl + 2) * NUNITS])
                if xl % 4 == 0:
                    w4 = wk.tile([128, 4 * Ncol], BF16, tag="w4")
                    nc.scalar.dma_start(
                        out=w4[:], in_=w_d[:, xl * Ncol:(xl + 4) * Ncol])
                g = gp.tile([128, Ncol], U32, tag="g")
                nc.gpsimd.ap_gather(
                    g[:], tb[:, (xl % 2) * NUNITS:(xl % 2 + 1) * NUNITS],
                    idx_sb[:, xl * NI16:(xl + 1) * NI16],
                    channels=128, num_elems=NUNITS, d=1, num_idxs=Ncol)
                prod = pp.tile([128, Ncol, 2], BF16, tag="prod")
                gv = g[:].bitcast(BF16).rearrange("p (n f) -> p n f", n=Ncol, f=2)
                wv = w4[:, (xl % 4) * Ncol:(xl % 4 + 1) * Ncol]
                nc.vector.tensor_tensor(
                    prod[:], gv,
                    wv.unsqueeze(-1).broadcast_to([128, Ncol, 2]),
                    mybir.AluOpType.mult)
                ps = psp.tile([48, 512 * SPX], F32, tag="ps")
                for t in range(CPX):
                    sgrp, q = t // 3, t % 3
                    nc.tensor.matmul(
                        ps[0:48, 512 * sgrp:512 * sgrp + 512],
                        lhsT=s_sb[:, 48 * q:48 * q + 48],
                        rhs=prod[:, 256 * t:256 * t + 256, :],
                        start=(q == 0), stop=(q == 2 or t == CPX - 1))
                osb = op.tile([48, 512 * SPX], BF16, tag="osb")
                nc.scalar.copy(osb[:], ps[0:48, :])
                nc.sync.dma_start(out=out_d[48 * xl:48 * xl + 48, :], in_=osb[:])
    nc.compile()
    return nc


def kernel(c0, c1, c2, table):
    c0 = np.asarray(c0, np.float32)
    c1 = np.asarray(c1, np.float32)
    c2 = np.asarray(c2, np.float32)
    table = np.asarray(table, np.float32)
    N = c0.shape[0]

    xs = [a * np.float32(RES - 1) for a in (c0, c1, c2)]
    i0 = [np.clip(np.floor(x), 0, RES - 2).astype(np.int32) for x in xs]
    fr = [x - i.astype(np.float32) for x, i in zip(xs, i0)]

    W8 = np.empty((N, 8), np.float32)
    for a in range(8):
        dx, dy, dz = (a >> 2) & 1, (a >> 1) & 1, a & 1
        W8[:, a] = ((fr[0] if dx else 1.0 - fr[0])
                    * (fr[1] if dy else 1.0 - fr[1])
                    * (fr[2] if dz else 1.0 - fr[2]))

    core = i0[0] >> 4
    xloc = i0[0] & 15
    y, z = i0[1], i0[2]
    zc = z & 7
    zblk = z >> 3
    cid = y * 16 + zblk
    grp = (core * 16 + xloc) * 8 + zc
    NG = NCORES * XL * 8

    cnt = np.zeros((NG, NGC), np.int32)
    np.add.at(cnt, (grp, cid), 1)

    order_cells = np.argsort(-cnt, axis=1, kind="stable")
    A = order_cells[:, 0::2]
    B = order_cells[:, 1::2]
    m = np.take_along_axis(cnt, A, axis=1)       # na >= nb
    off = np.zeros((NG, NUNITS), np.int64)
    off[:, 1:] = np.cumsum(m, axis=1)[:, :-1]
    Ncol = int(m.sum(axis=1).max())
    Ncol = ((Ncol + 767) // 768) * 768
    CPX = Ncol // 256
    SPX = (CPX + 2) // 3

    unit_of = np.zeros((NG, NGC), np.int32)
    bp_of = np.zeros((NG, NGC), np.int8)
    gi = np.arange(NG)[:, None]
    unit_of[gi, A] = np.arange(NUNITS)[None, :]
    unit_of[gi, B] = np.arange(NUNITS)[None, :]
    bp_of[gi, A] = 0
    bp_of[gi, B] = 1

    key = grp.astype(np.int64) * NGC + cid
    order = np.argsort(key, kind="stable")
    ks = key[order]
    newrun = np.ones(N, bool)
    newrun[1:] = ks[1:] != ks[:-1]
    runstart = np.flatnonzero(newrun)
    rid = np.cumsum(newrun) - 1
    rank = np.empty(N, np.int64)
    rank[order] = np.arange(N) - runstart[rid]

    unit_pt = unit_of[grp, cid].astype(np.int64)
    bp_pt = bp_of[grp, cid].astype(np.int64)
    col_pt = off[grp, unit_pt] + rank

    # idx tiles [8, 128, XL*Ncol/16] int16
    idx_tiles = np.zeros((NCORES, 128, XL * Ncol // 16), np.int16)
    seq = np.zeros((NG, Ncol), np.int16)
    units16 = np.arange(NUNITS, dtype=np.int16)
    for g in range(NG):
        s = np.repeat(units16, m[g])
        seq[g, :len(s)] = s
    seq = seq.reshape(NCORES, XL, 8, Ncol // 16, 16)
    for c in range(NCORES):
        for xl in range(XL):
            for zcl in range(8):
                idx_tiles[c, 16 * zcl:16 * zcl + 16,
                          xl * (Ncol // 16):(xl + 1) * (Ncol // 16)] = \
                    seq[c, xl, zcl].T

    # w tiles [8, 128, XL*Ncol] bf16
    w_tiles = np.zeros((NCORES, 128, XL * Ncol), ml_dtypes.bfloat16)
    lane_base = (zc * 16 + bp_pt * 8).astype(np.int64)
    gcol = xloc * Ncol + col_pt
    W8b = W8.astype(ml_dtypes.bfloat16)
    for a in range(8):
        w_tiles[core, lane_base + a, gcol] = W8b[:, a]

    # data tiles [8, 128, XL*1024] u32
    packed = _pack_table(table)
    data_tiles = np.zeros((NCORES, 128, XL * NUNITS), np.uint32)
    AB = np.stack([A, B], axis=1).reshape(NCORES, XL, 8, 2, NUNITS)
    for c in range(NCORES):
        for xl in range(XL):
            xg = 16 * c + xl
            for zcl in range(8):
                for bp in range(2):
                    cids = AB[c, xl, zcl, bp]
                    yy = cids >> 4
                    zz = (cids & 15) * 8 + zcl
                    data_tiles[c, 16 * zcl + 8 * bp:16 * zcl + 8 * bp + 8,
                               xl * NUNITS:(xl + 1) * NUNITS] = \
                        packed[xg, yy, zz, :].T

    S = np.zeros((128, 144), ml_dtypes.bfloat16)
    p = np.arange(128)
    pzc, pbp = p >> 4, (p >> 3) & 1
    for q in range(3):
        S[p, 48 * q + 16 * q + 2 * pzc + pbp] = 1.0

    nc = build_kernel(Ncol, SPX, CPX)
    _LAST["nc"] = nc

    in_maps = [{"data": data_tiles[c], "idx": idx_tiles[c],
                "w": w_tiles[c].view(np.uint16), "s": S.view(np.uint16)}
               for c in range(NCORES)]
    res = run_bass_kernel_spmd(nc, in_maps, core_ids=list(range(NCORES)))

    # unpack
    t = col_pt // 256
    rows = 48 * xloc + 16 * (t % 3) + 2 * zc + bp_pt
    cols0 = 512 * (t // 3) + (col_pt % 256) * 2
    out_full = np.empty((N, 2), np.float32)
    allout = np.stack([np.asarray(res.results[c]["out"]) for c in range(NCORES)])
    if allout.dtype == np.uint16:
        allout = allout.view(ml_dtypes.bfloat16)
    allout = allout.astype(np.float32)
    out_full[:, 0] = allout[core, rows, cols0]
    out_full[:, 1] = allout[core, rows, cols0 + 1]
    return out_full


# revision 8
# speedup vs baseline: 3.5990x; 1.1151x over previous
"""Trilerp kernel v4: ap_gather expansion + PE corner-reduce.

Per core (x-slab of 16): table packed bf16-pair-per-u32 into lanes
[128 = 8 z-class x 2 half x 8 corner], free = cell-pair unit. Host pairs
same-class cells by count (near-zero padding) and ships per-point corner
weights in lane layout. Device: ap_gather (Pool, one free-elem per point,
no DMA descriptors) -> DVE broadcast-mult -> PE matmul corner-reduce with
3 row-block stationaries accumulating into one PSUM bank -> Act drain to
bf16 -> DMA out. No SWDGE gathers at all.
"""
import sys
sys.path.insert(0, '/opt/trn_rl_repo')
import numpy as np
import ml_dtypes

import concourse.bass as bass
import concourse.mybir as mybir
from concourse import bacc
from concourse.tile import TileContext
from concourse.bass_utils import run_bass_kernel_spmd
from concourse.library_config import ap_gather as ap_gather_lib

BF16 = mybir.dt.bfloat16
U32 = mybir.dt.uint32
F32 = mybir.dt.float32
I16 = mybir.dt.int16

RES = 128
NCORES = 8
XL = 16               # x-values per core
NGC = 2048            # cells per (core, x_loc, z-class)
NUNITS = 1024         # cell pairs per (core, x_loc, z-class)
_LAST = {}


def _pack_table(table):
    """[128,128,128,2] f32 -> corner-packed u32 [x,y,z,8a] (bf16 f0|f1<<16)."""
    Tb = table.astype(ml_dtypes.bfloat16).view(np.uint16).astype(np.uint32)
    packed = np.empty((RES, RES, RES, 8), np.uint32)
    ip = np.minimum(np.arange(RES) + 1, RES - 1)
    for a in range(8):
        dx, dy, dz = (a >> 2) & 1, (a >> 1) & 1, a & 1
        V = Tb[ip if dx else slice(None)]
        V = V[:, ip if dy else slice(None)]
        V = V[:, :, ip if dz else slice(None)]
        packed[:, :, :, a] = V[..., 0] | (V[..., 1] << 16)
    return packed


def build_kernel(Ncol, SPX, CPX, ND):
    NS = Ncol - ND
    nc = bacc.Bacc("TRN2", target_bir_lowering=False, debug=False,
                   num_devices=NCORES)
    data_d = nc.dram_tensor("data", [128, XL * NUNITS], U32, kind="ExternalInput")
    idx_d = nc.dram_tensor("idx", [128, XL * Ncol // 16], I16, kind="ExternalInput")
    wd_d = nc.dram_tensor("wd", [128, XL * ND * 2], BF16, kind="ExternalInput")
    ws_d = nc.dram_tensor("ws", [128, XL * NS], BF16, kind="ExternalInput")
    s_d = nc.dram_tensor("s", [128, 144], BF16, kind="ExternalInput")
    out_d = nc.dram_tensor("out", [48 * XL, 512 * SPX], BF16, kind="ExternalOutput")
    with TileContext(nc) as tc:
        with tc.tile_pool(name="io", bufs=1) as io, \
             tc.tile_pool(name="tbl", bufs=3) as tbl, \
             tc.tile_pool(name="wk", bufs=2) as wk, \
             tc.tile_pool(name="gp", bufs=2) as gp, \
             tc.tile_pool(name="pp", bufs=2) as pp, \
             tc.tile_pool(name="op", bufs=2) as op, \
             tc.psum_pool(name="ps", bufs=2) as psp:
            nc.gpsimd.load_library(ap_gather_lib)
            s_sb = io.tile([128, 144], BF16, tag="s")
            nc.sync.dma_start(out=s_sb[:], in_=s_d[:])
            idx_sb = io.tile([128, XL * Ncol // 16], I16, tag="idx")
            nc.sync.dma_start(out=idx_sb[:], in_=idx_d[:])
            NI16 = Ncol // 16
            for xl in range(XL):
                if xl % 2 == 0:
                    tb = tbl.tile([128, 2 * NUNITS], U32, tag="tb")
                    nc.sync.dma_start(
                        out=tb[:],
                        in_=data_d[:, xl * NUNITS:(xl + 2) * NUNITS])
                if xl % 4 == 0:
                    wd4 = wk.tile([128, 4 * ND * 2], BF16, tag="wd4")
                    nc.scalar.dma_start(
                        out=wd4[:], in_=wd_d[:, xl * ND * 2:(xl + 4) * ND * 2])
                    ws4 = wk.tile([128, 4 * NS], BF16, tag="ws4")
                    nc.scalar.dma_start(
                        out=ws4[:], in_=ws_d[:, xl * NS:(xl + 4) * NS])
                g = gp.tile([128, Ncol], U32, tag="g")
                nc.gpsimd.ap_gather(
                    g[:], tb[:, (xl % 2) * NUNITS:(xl % 2 + 1) * NUNITS],
                    idx_sb[:, xl * NI16:(xl + 1) * NI16],
                    channels=128, num_elems=NUNITS, d=1, num_idxs=Ncol)
                prod = pp.tile([128, Ncol, 2], BF16, tag="prod")
                gv = g[:].bitcast(BF16).rearrange("p (n f) -> p n f", n=Ncol, f=2)
                wdv = wd4[:, (xl % 4) * ND * 2:(xl % 4 + 1) * ND * 2] \
                    .rearrange("p (n f) -> p n f", n=ND, f=2)
                nc.vector.tensor_tensor(
                    prod[:, 0:ND, :], gv[:, 0:ND, :], wdv,
                    mybir.AluOpType.mult)
                wsv = ws4[:, (xl % 4) * NS:(xl % 4 + 1) * NS]
                nc.vector.tensor_tensor(
                    prod[:, ND:Ncol, :], gv[:, ND:Ncol, :],
                    wsv.unsqueeze(-1).broadcast_to([128, NS, 2]),
                    mybir.AluOpType.mult)
                ps = psp.tile([48, 512 * SPX], F32, tag="ps")
                for t in range(CPX):
                    sgrp, q = t // 3, t % 3
                    nc.tensor.matmul(
                        ps[0:48, 512 * sgrp:512 * sgrp + 512],
                        lhsT=s_sb[:, 48 * q:48 * q + 48],
                        rhs=prod[:, 256 * t:256 * t + 256, :],
                        start=(q == 0), stop=(q == 2 or t == CPX - 1))
                osb = op.tile([48, 512 * SPX], BF16, tag="osb")
                nc.scalar.copy(osb[:], ps[0:48, :])
                nc.sync.dma_start(out=out_d[48 * xl:48 * xl + 48, :], in_=osb[:])
    nc.compile()
    return nc


def kernel(c0, c1, c2, table):
    c0 = np.asarray(c0, np.float32)
    c1 = np.asarray(c1, np.float32)
    c2 = np.asarray(c2, np.float32)
    table = np.asarray(table, np.float32)
    N = c0.shape[0]

    xs = [a * np.float32(RES - 1) for a in (c0, c1, c2)]
    i0 = [np.clip(np.floor(x), 0, RES - 2).astype(np.int32) for x in xs]
    fr = [x - i.astype(np.float32) for x, i in zip(xs, i0)]

    W8 = np.empty((N, 8), np.float32)
    for a in range(8):
        dx, dy, dz = (a >> 2) & 1, (a >> 1) & 1, a & 1
        W8[:, a] = ((fr[0] if dx else 1.0 - fr[0])
                    * (fr[1] if dy else 1.0 - fr[1])
                    * (fr[2] if dz else 1.0 - fr[2]))

    core = i0[0] >> 4
    xloc = i0[0] & 15
    y, z = i0[1], i0[2]
    zc = z & 7
    zblk = z >> 3
    cid = y * 16 + zblk
    grp = (core * 16 + xloc) * 8 + zc
    NG = NCORES * XL * 8

    cnt = np.zeros((NG, NGC), np.int32)
    np.add.at(cnt, (grp, cid), 1)

    order_cells = np.argsort(-cnt, axis=1, kind="stable")
    A = order_cells[:, 0::2]
    B = order_cells[:, 1::2]
    m = np.take_along_axis(cnt, A, axis=1)       # na >= nb
    off = np.zeros((NG, NUNITS), np.int64)
    off[:, 1:] = np.cumsum(m, axis=1)[:, :-1]
    Ncol = int(m.sum(axis=1).max())
    Ncol = ((Ncol + 767) // 768) * 768
    CPX = Ncol // 256
    SPX = (CPX + 2) // 3
    ND = max(256, (Ncol // 3 // 256) * 256)   # dup-w region (fast mult)
    NS = Ncol - ND

    unit_of = np.zeros((NG, NGC), np.int32)
    bp_of = np.zeros((NG, NGC), np.int8)
    gi = np.arange(NG)[:, None]
    unit_of[gi, A] = np.arange(NUNITS)[None, :]
    unit_of[gi, B] = np.arange(NUNITS)[None, :]
    bp_of[gi, A] = 0
    bp_of[gi, B] = 1

    key = grp.astype(np.int64) * NGC + cid
    order = np.argsort(key, kind="stable")
    ks = key[order]
    newrun = np.ones(N, bool)
    newrun[1:] = ks[1:] != ks[:-1]
    runstart = np.flatnonzero(newrun)
    rid = np.cumsum(newrun) - 1
    rank = np.empty(N, np.int64)
    rank[order] = np.arange(N) - runstart[rid]

    unit_pt = unit_of[grp, cid].astype(np.int64)
    bp_pt = bp_of[grp, cid].astype(np.int64)
    col_pt = off[grp, unit_pt] + rank

    # idx tiles [8, 128, XL*Ncol/16] int16
    idx_tiles = np.zeros((NCORES, 128, XL * Ncol // 16), np.int16)
    seq = np.zeros((NG, Ncol), np.int16)
    units16 = np.arange(NUNITS, dtype=np.int16)
    for g in range(NG):
        s = np.repeat(units16, m[g])
        seq[g, :len(s)] = s
    seq = seq.reshape(NCORES, XL, 8, Ncol // 16, 16)
    for c in range(NCORES):
        for xl in range(XL):
            for zcl in range(8):
                idx_tiles[c, 16 * zcl:16 * zcl + 16,
                          xl * (Ncol // 16):(xl + 1) * (Ncol // 16)] = \
                    seq[c, xl, zcl].T

    # w tiles: dup region [8, 128, XL*ND*2], bcast region [8, 128, XL*NS]
    wd_tiles = np.zeros((NCORES, 128, XL * ND * 2), ml_dtypes.bfloat16)
    ws_tiles = np.zeros((NCORES, 128, XL * NS), ml_dtypes.bfloat16)
    lane_base = (zc * 16 + bp_pt * 8).astype(np.int64)
    W8b = W8.astype(ml_dtypes.bfloat16)
    in_d = col_pt < ND
    in_s = ~in_d
    gcol_d = (xloc[in_d] * ND + col_pt[in_d]) * 2
    gcol_s = xloc[in_s] * NS + (col_pt[in_s] - ND)
    for a in range(8):
        wd_tiles[core[in_d], lane_base[in_d] + a, gcol_d] = W8b[in_d, a]
        wd_tiles[core[in_d], lane_base[in_d] + a, gcol_d + 1] = W8b[in_d, a]
        ws_tiles[core[in_s], lane_base[in_s] + a, gcol_s] = W8b[in_s, a]

    # data tiles [8, 128, XL*1024] u32
    packed = _pack_table(table)
    data_tiles = np.zeros((NCORES, 128, XL * NUNITS), np.uint32)
    AB = np.stack([A, B], axis=1).reshape(NCORES, XL, 8, 2, NUNITS)
    for c in range(NCORES):
        for xl in range(XL):
            xg = 16 * c + xl
            for zcl in range(8):
                for bp in range(2):
                    cids = AB[c, xl, zcl, bp]
                    yy = cids >> 4
                    zz = (cids & 15) * 8 + zcl
                    data_tiles[c, 16 * zcl + 8 * bp:16 * zcl + 8 * bp + 8,
                               xl * NUNITS:(xl + 1) * NUNITS] = \
                        packed[xg, yy, zz, :].T

    S = np.zeros((128, 144), ml_dtypes.bfloat16)
    p = np.arange(128)
    pzc, pbp = p >> 4, (p >> 3) & 1
    for q in range(3):
        S[p, 48 * q + 16 * q + 2 * pzc + pbp] = 1.0

    nc = build_kernel(Ncol, SPX, CPX, ND)
    _LAST["nc"] = nc

    in_maps = [{"data": data_tiles[c], "idx": idx_tiles[c],
                "wd": wd_tiles[c].view(np.uint16),
                "ws": ws_tiles[c].view(np.uint16), "s": S.view(np.uint16)}
               for c in range(NCORES)]
    res = run_bass_kernel_spmd(nc, in_maps, core_ids=list(range(NCORES)))

    # unpack
    t = col_pt // 256
    rows = 48 * xloc + 16 * (t % 3) + 2 * zc + bp_pt
    cols0 = 512 * (t // 3) + (col_pt % 256) * 2
    out_full = np.empty((N, 2), np.float32)
    allout = np.stack([np.asarray(res.results[c]["out"]) for c in range(NCORES)])
    if allout.dtype == np.uint16:
        allout = allout.view(ml_dtypes.bfloat16)
    allout = allout.astype(np.float32)
    out_full[:, 0] = allout[core, rows, cols0]
    out_full[:, 1] = allout[core, rows, cols0 + 1]
    return out_full


# revision 9
# speedup vs baseline: 3.8019x; 1.0564x over previous
"""Trilerp kernel v4: ap_gather expansion + PE corner-reduce.

Per core (x-slab of 16): table packed bf16-pair-per-u32 into lanes
[128 = 8 z-class x 2 half x 8 corner], free = cell-pair unit. Host pairs
same-class cells by count (near-zero padding) and ships per-point corner
weights in lane layout. Device: ap_gather (Pool, one free-elem per point,
no DMA descriptors) -> DVE broadcast-mult -> PE matmul corner-reduce with
3 row-block stationaries accumulating into one PSUM bank -> Act drain to
bf16 -> DMA out. No SWDGE gathers at all.
"""
import sys
sys.path.insert(0, '/opt/trn_rl_repo')
import numpy as np
import ml_dtypes

import concourse.bass as bass
import concourse.mybir as mybir
from concourse import bacc
from concourse.tile import TileContext
from concourse.bass_utils import run_bass_kernel_spmd
from concourse.library_config import ap_gather as ap_gather_lib

BF16 = mybir.dt.bfloat16
U32 = mybir.dt.uint32
F32 = mybir.dt.float32
I16 = mybir.dt.int16

RES = 128
NCORES = 8
XL = 16               # x-values per core
NGC = 2048            # cells per (core, x_loc, z-class)
NUNITS = 1024         # cell pairs per (core, x_loc, z-class)
_LAST = {}


def _pack_table(table):
    """[128,128,128,2] f32 -> corner-packed u32 [x,y,z,8a] (bf16 f0|f1<<16)."""
    Tb = table.astype(ml_dtypes.bfloat16).view(np.uint16).astype(np.uint32)
    packed = np.empty((RES, RES, RES, 8), np.uint32)
    ip = np.minimum(np.arange(RES) + 1, RES - 1)
    for a in range(8):
        dx, dy, dz = (a >> 2) & 1, (a >> 1) & 1, a & 1
        V = Tb[ip if dx else slice(None)]
        V = V[:, ip if dy else slice(None)]
        V = V[:, :, ip if dz else slice(None)]
        packed[:, :, :, a] = V[..., 0] | (V[..., 1] << 16)
    return packed


def build_kernel(Ncol, SPX, CPX, ND):
    NS = Ncol - ND
    nc = bacc.Bacc("TRN2", target_bir_lowering=False, debug=False,
                   num_devices=NCORES)
    data_d = nc.dram_tensor("data", [128, XL * NUNITS], U32, kind="ExternalInput")
    idx_d = nc.dram_tensor("idx", [128, XL * Ncol // 16], I16, kind="ExternalInput")
    wd_d = nc.dram_tensor("wd", [128, XL * ND * 2], BF16, kind="ExternalInput")
    ws_d = nc.dram_tensor("ws", [128, XL * NS], BF16, kind="ExternalInput")
    s_d = nc.dram_tensor("s", [128, 144], BF16, kind="ExternalInput")
    out_d = nc.dram_tensor("out", [48 * XL, 512 * SPX], BF16, kind="ExternalOutput")
    with TileContext(nc) as tc:
        with tc.tile_pool(name="io", bufs=1) as io, \
             tc.tile_pool(name="tbl", bufs=3) as tbl, \
             tc.tile_pool(name="wk", bufs=2) as wk, \
             tc.tile_pool(name="gp", bufs=2) as gp, \
             tc.tile_pool(name="pp", bufs=2) as pp, \
             tc.tile_pool(name="op", bufs=2) as op, \
             tc.psum_pool(name="ps", bufs=2) as psp:
            nc.gpsimd.load_library(ap_gather_lib)
            s_sb = io.tile([128, 144], BF16, tag="s")
            nc.sync.dma_start(out=s_sb[:], in_=s_d[:])
            idx_sb = io.tile([128, XL * Ncol // 16], I16, tag="idx")
            nc.sync.dma_start(out=idx_sb[:], in_=idx_d[:])
            NI16 = Ncol // 16
            H0 = (CPX // 2) * 256         # first half (256-aligned)
            H1 = Ncol - H0
            assert ND <= H0
            for xl in range(XL):
                tb = tbl.tile([128, NUNITS], U32, tag="tb")
                nc.sync.dma_start(
                    out=tb[:], in_=data_d[:, xl * NUNITS:(xl + 1) * NUNITS])
                if xl % 2 == 0:
                    wd2 = wk.tile([128, 2 * ND * 2], BF16, tag="wd2")
                    nc.sync.dma_start(
                        out=wd2[:], in_=wd_d[:, xl * ND * 2:(xl + 2) * ND * 2])
                    ws2 = wk.tile([128, 2 * NS], BF16, tag="ws2")
                    nc.sync.dma_start(
                        out=ws2[:], in_=ws_d[:, xl * NS:(xl + 2) * NS])
                wdv = wd2[:, (xl % 2) * ND * 2:(xl % 2 + 1) * ND * 2] \
                    .rearrange("p (n f) -> p n f", n=ND, f=2)
                wsv = ws2[:, (xl % 2) * NS:(xl % 2 + 1) * NS]
                ibase = xl * NI16
                g0 = gp.tile([128, H0], U32, tag="g0")
                nc.gpsimd.ap_gather(
                    g0[:], tb[:], idx_sb[:, ibase:ibase + H0 // 16],
                    channels=128, num_elems=NUNITS, d=1, num_idxs=H0)
                g1 = gp.tile([128, H1], U32, tag="g1")
                nc.gpsimd.ap_gather(
                    g1[:], tb[:], idx_sb[:, ibase + H0 // 16:ibase + NI16],
                    channels=128, num_elems=NUNITS, d=1, num_idxs=H1)
                prod0 = pp.tile([128, H0, 2], BF16, tag="prod0")
                gv0 = g0[:].bitcast(BF16).rearrange("p (n f) -> p n f", n=H0, f=2)
                nc.vector.tensor_tensor(
                    prod0[:, 0:ND, :], gv0[:, 0:ND, :], wdv,
                    mybir.AluOpType.mult)
                if H0 > ND:
                    nc.vector.tensor_tensor(
                        prod0[:, ND:H0, :], gv0[:, ND:H0, :],
                        wsv[:, 0:H0 - ND].unsqueeze(-1)
                            .broadcast_to([128, H0 - ND, 2]),
                        mybir.AluOpType.mult)
                prod1 = pp.tile([128, H1, 2], BF16, tag="prod1")
                gv1 = g1[:].bitcast(BF16).rearrange("p (n f) -> p n f", n=H1, f=2)
                nc.vector.tensor_tensor(
                    prod1[:], gv1,
                    wsv[:, H0 - ND:Ncol - ND].unsqueeze(-1)
                        .broadcast_to([128, H1, 2]),
                    mybir.AluOpType.mult)
                ps = psp.tile([48, 512 * SPX], F32, tag="ps")
                for t in range(CPX):
                    sgrp, q = t // 3, t % 3
                    if (t + 1) * 256 <= H0:
                        rhs = prod0[:, 256 * t:256 * t + 256, :]
                    else:
                        u = t - H0 // 256
                        rhs = prod1[:, 256 * u:256 * u + 256, :]
                    nc.tensor.matmul(
                        ps[0:48, 512 * sgrp:512 * sgrp + 512],
                        lhsT=s_sb[:, 48 * q:48 * q + 48],
                        rhs=rhs,
                        start=(q == 0), stop=(q == 2 or t == CPX - 1))
                osb = op.tile([48, 512 * SPX], BF16, tag="osb")
                nc.scalar.copy(osb[:], ps[0:48, :])
                nc.sync.dma_start(out=out_d[48 * xl:48 * xl + 48, :], in_=osb[:])
    nc.compile()
    return nc


def kernel(c0, c1, c2, table):
    c0 = np.asarray(c0, np.float32)
    c1 = np.asarray(c1, np.float32)
    c2 = np.asarray(c2, np.float32)
    table = np.asarray(table, np.float32)
    N = c0.shape[0]

    xs = [a * np.float32(RES - 1) for a in (c0, c1, c2)]
    i0 = [np.clip(np.floor(x), 0, RES - 2).astype(np.int32) for x in xs]
    fr = [x - i.astype(np.float32) for x, i in zip(xs, i0)]

    W8 = np.empty((N, 8), np.float32)
    for a in range(8):
        dx, dy, dz = (a >> 2) & 1, (a >> 1) & 1, a & 1
        W8[:, a] = ((fr[0] if dx else 1.0 - fr[0])
                    * (fr[1] if dy else 1.0 - fr[1])
                    * (fr[2] if dz else 1.0 - fr[2]))

    core = i0[0] >> 4
    xloc = i0[0] & 15
    y, z = i0[1], i0[2]
    zc = z & 7
    zblk = z >> 3
    cid = y * 16 + zblk
    grp = (core * 16 + xloc) * 8 + zc
    NG = NCORES * XL * 8

    cnt = np.zeros((NG, NGC), np.int32)
    np.add.at(cnt, (grp, cid), 1)

    order_cells = np.argsort(-cnt, axis=1, kind="stable")
    A = order_cells[:, 0::2]
    B = order_cells[:, 1::2]
    m = np.take_along_axis(cnt, A, axis=1)       # na >= nb
    off = np.zeros((NG, NUNITS), np.int64)
    off[:, 1:] = np.cumsum(m, axis=1)[:, :-1]
    Ncol = int(m.sum(axis=1).max())
    Ncol = ((Ncol + 767) // 768) * 768
    CPX = Ncol // 256
    SPX = (CPX + 2) // 3
    ND = max(256, (Ncol // 3 // 256) * 256)   # dup-w region (fast mult)
    NS = Ncol - ND

    unit_of = np.zeros((NG, NGC), np.int32)
    bp_of = np.zeros((NG, NGC), np.int8)
    gi = np.arange(NG)[:, None]
    unit_of[gi, A] = np.arange(NUNITS)[None, :]
    unit_of[gi, B] = np.arange(NUNITS)[None, :]
    bp_of[gi, A] = 0
    bp_of[gi, B] = 1

    key = grp.astype(np.int64) * NGC + cid
    order = np.argsort(key, kind="stable")
    ks = key[order]
    newrun = np.ones(N, bool)
    newrun[1:] = ks[1:] != ks[:-1]
    runstart = np.flatnonzero(newrun)
    rid = np.cumsum(newrun) - 1
    rank = np.empty(N, np.int64)
    rank[order] = np.arange(N) - runstart[rid]

    unit_pt = unit_of[grp, cid].astype(np.int64)
    bp_pt = bp_of[grp, cid].astype(np.int64)
    col_pt = off[grp, unit_pt] + rank

    # idx tiles [8, 128, XL*Ncol/16] int16
    idx_tiles = np.zeros((NCORES, 128, XL * Ncol // 16), np.int16)
    seq = np.zeros((NG, Ncol), np.int16)
    units16 = np.arange(NUNITS, dtype=np.int16)
    for g in range(NG):
        s = np.repeat(units16, m[g])
        seq[g, :len(s)] = s
    seq = seq.reshape(NCORES, XL, 8, Ncol // 16, 16)
    for c in range(NCORES):
        for xl in range(XL):
            for zcl in range(8):
                idx_tiles[c, 16 * zcl:16 * zcl + 16,
                          xl * (Ncol // 16):(xl + 1) * (Ncol // 16)] = \
                    seq[c, xl, zcl].T

    # w tiles: dup region [8, 128, XL*ND*2], bcast region [8, 128, XL*NS]
    wd_tiles = np.zeros((NCORES, 128, XL * ND * 2), ml_dtypes.bfloat16)
    ws_tiles = np.zeros((NCORES, 128, XL * NS), ml_dtypes.bfloat16)
    lane_base = (zc * 16 + bp_pt * 8).astype(np.int64)
    W8b = W8.astype(ml_dtypes.bfloat16)
    in_d = col_pt < ND
    in_s = ~in_d
    gcol_d = (xloc[in_d] * ND + col_pt[in_d]) * 2
    gcol_s = xloc[in_s] * NS + (col_pt[in_s] - ND)
    for a in range(8):
        wd_tiles[core[in_d], lane_base[in_d] + a, gcol_d] = W8b[in_d, a]
        wd_tiles[core[in_d], lane_base[in_d] + a, gcol_d + 1] = W8b[in_d, a]
        ws_tiles[core[in_s], lane_base[in_s] + a, gcol_s] = W8b[in_s, a]

    # data tiles [8, 128, XL*1024] u32
    packed = _pack_table(table)
    data_tiles = np.zeros((NCORES, 128, XL * NUNITS), np.uint32)
    AB = np.stack([A, B], axis=1).reshape(NCORES, XL, 8, 2, NUNITS)
    for c in range(NCORES):
        for xl in range(XL):
            xg = 16 * c + xl
            for zcl in range(8):
                for bp in range(2):
                    cids = AB[c, xl, zcl, bp]
                    yy = cids >> 4
                    zz = (cids & 15) * 8 + zcl
                    data_tiles[c, 16 * zcl + 8 * bp:16 * zcl + 8 * bp + 8,
                               xl * NUNITS:(xl + 1) * NUNITS] = \
                        packed[xg, yy, zz, :].T

    S = np.zeros((128, 144), ml_dtypes.bfloat16)
    p = np.arange(128)
    pzc, pbp = p >> 4, (p >> 3) & 1
    for q in range(3):
        S[p, 48 * q + 16 * q + 2 * pzc + pbp] = 1.0

    nc = build_kernel(Ncol, SPX, CPX, ND)
    _LAST["nc"] = nc

    in_maps = [{"data": data_tiles[c], "idx": idx_tiles[c],
                "wd": wd_tiles[c].view(np.uint16),
                "ws": ws_tiles[c].view(np.uint16), "s": S.view(np.uint16)}
               for c in range(NCORES)]
    res = run_bass_kernel_spmd(nc, in_maps, core_ids=list(range(NCORES)))

    # unpack
    t = col_pt // 256
    rows = 48 * xloc + 16 * (t % 3) + 2 * zc + bp_pt
    cols0 = 512 * (t // 3) + (col_pt % 256) * 2
    out_full = np.empty((N, 2), np.float32)
    allout = np.stack([np.asarray(res.results[c]["out"]) for c in range(NCORES)])
    if allout.dtype == np.uint16:
        allout = allout.view(ml_dtypes.bfloat16)
    allout = allout.astype(np.float32)
    out_full[:, 0] = allout[core, rows, cols0]
    out_full[:, 1] = allout[core, rows, cols0 + 1]
    return out_full


# revision 14
# speedup vs baseline: 4.2983x; 1.1306x over previous
"""Trilerp kernel v4: ap_gather expansion + PE corner-reduce.

Per core (x-slab of 16): table packed bf16-pair-per-u32 into lanes
[128 = 8 z-class x 2 half x 8 corner], free = cell-pair unit. Host pairs
same-class cells by count (near-zero padding) and ships per-point corner
weights in lane layout. Device: ap_gather (Pool, one free-elem per point,
no DMA descriptors) -> DVE broadcast-mult -> PE matmul corner-reduce with
3 row-block stationaries accumulating into one PSUM bank -> Act drain to
bf16 -> DMA out. No SWDGE gathers at all.
"""
import sys
sys.path.insert(0, '/opt/trn_rl_repo')
import numpy as np
import ml_dtypes

import concourse.bass as bass
import concourse.mybir as mybir
from concourse import bacc
from concourse.tile import TileContext
from concourse.bass_utils import run_bass_kernel_spmd
from concourse.library_config import ap_gather as ap_gather_lib

BF16 = mybir.dt.bfloat16
U32 = mybir.dt.uint32
F32 = mybir.dt.float32
I16 = mybir.dt.int16

RES = 128
NCORES = 8
XL = 16               # x-values per core
NGC = 2048            # cells per (core, x_loc, z-class)
NUNITS = 1024         # cell pairs per (core, x_loc, z-class)
_LAST = {}


def _pack_table(table):
    """[128,128,128,2] f32 -> corner-packed u32 [x,y,z,8a] (bf16 f0|f1<<16)."""
    Tb = table.astype(ml_dtypes.bfloat16).view(np.uint16).astype(np.uint32)
    packed = np.empty((RES, RES, RES, 8), np.uint32)
    ip = np.minimum(np.arange(RES) + 1, RES - 1)
    for a in range(8):
        dx, dy, dz = (a >> 2) & 1, (a >> 1) & 1, a & 1
        V = Tb[ip if dx else slice(None)]
        V = V[:, ip if dy else slice(None)]
        V = V[:, :, ip if dz else slice(None)]
        packed[:, :, :, a] = V[..., 0] | (V[..., 1] << 16)
    return packed


def build_kernel(Ncol, SPX, CPX, ND):
    NS = Ncol - ND
    nc = bacc.Bacc("TRN2", target_bir_lowering=False, debug=False,
                   num_devices=NCORES)
    data_d = nc.dram_tensor("data", [128, XL * NUNITS], U32, kind="ExternalInput")
    idx_d = nc.dram_tensor("idx", [128, XL * Ncol // 16], I16, kind="ExternalInput")
    w_d = nc.dram_tensor("w", [128, XL * Ncol], BF16, kind="ExternalInput")
    s_d = nc.dram_tensor("s", [128, 144], BF16, kind="ExternalInput")
    out_d = nc.dram_tensor("out", [48 * XL, 512 * SPX], BF16, kind="ExternalOutput")
    with TileContext(nc) as tc:
        with tc.tile_pool(name="io", bufs=1) as io, \
             tc.tile_pool(name="tbl", bufs=3) as tbl, \
             tc.tile_pool(name="wk", bufs=2) as wk, \
             tc.tile_pool(name="gp", bufs=2) as gp, \
             tc.tile_pool(name="pp", bufs=2) as pp, \
             tc.tile_pool(name="op", bufs=2) as op, \
             tc.psum_pool(name="ps", bufs=2) as psp:
            nc.gpsimd.load_library(ap_gather_lib)
            s_sb = io.tile([128, 144], BF16, tag="s")
            nc.sync.dma_start(out=s_sb[:], in_=s_d[:])
            idx_sb = io.tile([128, XL * Ncol // 16], I16, tag="idx")
            nc.sync.dma_start(out=idx_sb[:], in_=idx_d[:])
            NI16 = Ncol // 16
            H0 = (CPX // 2) * 256         # first half (256-aligned)
            H1 = Ncol - H0
            for xl in range(XL):
                tb = tbl.tile([128, NUNITS], U32, tag="tb")
                nc.sync.dma_start(
                    out=tb[:], in_=data_d[:, xl * NUNITS:(xl + 1) * NUNITS])
                if xl % 2 == 0:
                    w2 = wk.tile([128, 2 * Ncol], BF16, tag="w2")
                    nc.sync.dma_start(
                        out=w2[:], in_=w_d[:, xl * Ncol:(xl + 2) * Ncol])
                wx = w2[:, (xl % 2) * Ncol:(xl % 2 + 1) * Ncol]
                wdup = gp.tile([128, ND, 2], BF16, tag="wdup")
                nc.scalar.copy(
                    wdup[:],
                    wx[:, 0:ND].unsqueeze(-1).broadcast_to([128, ND, 2]))
                wdv = wdup[:]
                wsv = wx[:, ND:Ncol]
                ibase = xl * NI16
                g0 = gp.tile([128, H0], U32, tag="g0")
                nc.gpsimd.ap_gather(
                    g0[:], tb[:], idx_sb[:, ibase:ibase + H0 // 16],
                    channels=128, num_elems=NUNITS, d=1, num_idxs=H0)
                g1 = gp.tile([128, H1], U32, tag="g1")
                nc.gpsimd.ap_gather(
                    g1[:], tb[:], idx_sb[:, ibase + H0 // 16:ibase + NI16],
                    channels=128, num_elems=NUNITS, d=1, num_idxs=H1)
                prod0 = pp.tile([128, H0, 2], BF16, tag="prod0")
                gv0 = g0[:].bitcast(BF16).rearrange("p (n f) -> p n f", n=H0, f=2)
                d0 = min(ND, H0)
                nc.vector.tensor_tensor(
                    prod0[:, 0:d0, :], gv0[:, 0:d0, :], wdv[:, 0:d0, :],
                    mybir.AluOpType.mult)
                if H0 > ND:
                    nc.vector.tensor_tensor(
                        prod0[:, ND:H0, :], gv0[:, ND:H0, :],
                        wsv[:, 0:H0 - ND].unsqueeze(-1)
                            .broadcast_to([128, H0 - ND, 2]),
                        mybir.AluOpType.mult)
                prod1 = pp.tile([128, H1, 2], BF16, tag="prod1")
                gv1 = g1[:].bitcast(BF16).rearrange("p (n f) -> p n f", n=H1, f=2)
                if ND > H0:
                    d1 = ND - H0
                    nc.vector.tensor_tensor(
                        prod1[:, 0:d1, :], gv1[:, 0:d1, :], wdv[:, H0:ND, :],
                        mybir.AluOpType.mult)
                    nc.vector.tensor_tensor(
                        prod1[:, d1:H1, :], gv1[:, d1:H1, :],
                        wsv.unsqueeze(-1).broadcast_to([128, Ncol - ND, 2]),
                        mybir.AluOpType.mult)
                else:
                    nc.vector.tensor_tensor(
                        prod1[:], gv1,
                        wsv[:, H0 - ND:Ncol - ND].unsqueeze(-1)
                            .broadcast_to([128, H1, 2]),
                        mybir.AluOpType.mult)
                ps = psp.tile([48, 512 * SPX], F32, tag="ps")
                for t in range(CPX):
                    sgrp, q = t // 3, t % 3
                    if (t + 1) * 256 <= H0:
                        rhs = prod0[:, 256 * t:256 * t + 256, :]
                    else:
                        u = t - H0 // 256
                        rhs = prod1[:, 256 * u:256 * u + 256, :]
                    nc.tensor.matmul(
                        ps[0:48, 512 * sgrp:512 * sgrp + 512],
                        lhsT=s_sb[:, 48 * q:48 * q + 48],
                        rhs=rhs,
                        start=(q == 0), stop=(q == 2 or t == CPX - 1))
                osb = op.tile([48, 512 * SPX], BF16, tag="osb")
                nc.scalar.copy(osb[:], ps[0:48, :])
                nc.sync.dma_start(out=out_d[48 * xl:48 * xl + 48, :], in_=osb[:])
    nc.compile()
    return nc


def kernel(c0, c1, c2, table):
    c0 = np.asarray(c0, np.float32)
    c1 = np.asarray(c1, np.float32)
    c2 = np.asarray(c2, np.float32)
    table = np.asarray(table, np.float32)
    N = c0.shape[0]

    xs = [a * np.float32(RES - 1) for a in (c0, c1, c2)]
    i0 = [np.clip(np.floor(x), 0, RES - 2).astype(np.int32) for x in xs]
    fr = [x - i.astype(np.float32) for x, i in zip(xs, i0)]

    W8 = np.empty((N, 8), np.float32)
    for a in range(8):
        dx, dy, dz = (a >> 2) & 1, (a >> 1) & 1, a & 1
        W8[:, a] = ((fr[0] if dx else 1.0 - fr[0])
                    * (fr[1] if dy else 1.0 - fr[1])
                    * (fr[2] if dz else 1.0 - fr[2]))

    core = i0[0] >> 4
    xloc = i0[0] & 15
    y, z = i0[1], i0[2]
    zc = z & 7
    zblk = z >> 3
    cid = y * 16 + zblk
    grp = (core * 16 + xloc) * 8 + zc
    NG = NCORES * XL * 8

    cnt = np.zeros((NG, NGC), np.int32)
    np.add.at(cnt, (grp, cid), 1)

    order_cells = np.argsort(-cnt, axis=1, kind="stable")
    A = order_cells[:, 0::2]
    B = order_cells[:, 1::2]
    m = np.take_along_axis(cnt, A, axis=1)       # na >= nb
    off = np.zeros((NG, NUNITS), np.int64)
    off[:, 1:] = np.cumsum(m, axis=1)[:, :-1]
    Ncol = int(m.sum(axis=1).max())
    Ncol = ((Ncol + 767) // 768) * 768
    CPX = Ncol // 256
    SPX = (CPX + 2) // 3
    ND = min(Ncol - 256, max(256, round(Ncol * 0.55 / 256) * 256))
    NS = Ncol - ND

    unit_of = np.zeros((NG, NGC), np.int32)
    bp_of = np.zeros((NG, NGC), np.int8)
    gi = np.arange(NG)[:, None]
    unit_of[gi, A] = np.arange(NUNITS)[None, :]
    unit_of[gi, B] = np.arange(NUNITS)[None, :]
    bp_of[gi, A] = 0
    bp_of[gi, B] = 1

    key = grp.astype(np.int64) * NGC + cid
    order = np.argsort(key, kind="stable")
    ks = key[order]
    newrun = np.ones(N, bool)
    newrun[1:] = ks[1:] != ks[:-1]
    runstart = np.flatnonzero(newrun)
    rid = np.cumsum(newrun) - 1
    rank = np.empty(N, np.int64)
    rank[order] = np.arange(N) - runstart[rid]

    unit_pt = unit_of[grp, cid].astype(np.int64)
    bp_pt = bp_of[grp, cid].astype(np.int64)
    col_pt = off[grp, unit_pt] + rank

    # idx tiles [8, 128, XL*Ncol/16] int16
    idx_tiles = np.zeros((NCORES, 128, XL * Ncol // 16), np.int16)
    seq = np.zeros((NG, Ncol), np.int16)
    units16 = np.arange(NUNITS, dtype=np.int16)
    for g in range(NG):
        s = np.repeat(units16, m[g])
        seq[g, :len(s)] = s
    seq = seq.reshape(NCORES, XL, 8, Ncol // 16, 16)
    for c in range(NCORES):
        for xl in range(XL):
            for zcl in range(8):
                idx_tiles[c, 16 * zcl:16 * zcl + 16,
                          xl * (Ncol // 16):(xl + 1) * (Ncol // 16)] = \
                    seq[c, xl, zcl].T

    # w tiles [8, 128, XL*Ncol] bf16 (compact; device duplicates [0, ND))
    w_tiles = np.zeros((NCORES, 128, XL * Ncol), ml_dtypes.bfloat16)
    lane_base = (zc * 16 + bp_pt * 8).astype(np.int64)
    W8b = W8.astype(ml_dtypes.bfloat16)
    gcol = xloc * Ncol + col_pt
    for a in range(8):
        w_tiles[core, lane_base + a, gcol] = W8b[:, a]

    # data tiles [8, 128, XL*1024] u32
    packed = _pack_table(table)
    data_tiles = np.zeros((NCORES, 128, XL * NUNITS), np.uint32)
    AB = np.stack([A, B], axis=1).reshape(NCORES, XL, 8, 2, NUNITS)
    for c in range(NCORES):
        for xl in range(XL):
            xg = 16 * c + xl
            for zcl in range(8):
                for bp in range(2):
                    cids = AB[c, xl, zcl, bp]
                    yy = cids >> 4
                    zz = (cids & 15) * 8 + zcl
                    data_tiles[c, 16 * zcl + 8 * bp:16 * zcl + 8 * bp + 8,
                               xl * NUNITS:(xl + 1) * NUNITS] = \
                        packed[xg, yy, zz, :].T

    S = np.zeros((128, 144), ml_dtypes.bfloat16)
    p = np.arange(128)
    pzc, pbp = p >> 4, (p >> 3) & 1
    for q in range(3):
        S[p, 48 * q + 16 * q + 2 * pzc + pbp] = 1.0

    nc = build_kernel(Ncol, SPX, CPX, ND)
    _LAST["nc"] = nc

    in_maps = [{"data": data_tiles[c], "idx": idx_tiles[c],
                "w": w_tiles[c].view(np.uint16), "s": S.view(np.uint16)}
               for c in range(NCORES)]
    res = run_bass_kernel_spmd(nc, in_maps, core_ids=list(range(NCORES)))

    # unpack
    t = col_pt // 256
    rows = 48 * xloc + 16 * (t % 3) + 2 * zc + bp_pt
    cols0 = 512 * (t // 3) + (col_pt % 256) * 2
    out_full = np.empty((N, 2), np.float32)
    allout = np.stack([np.asarray(res.results[c]["out"]) for c in range(NCORES)])
    if allout.dtype == np.uint16:
        allout = allout.view(ml_dtypes.bfloat16)
    allout = allout.astype(np.float32)
    out_full[:, 0] = allout[core, rows, cols0]
    out_full[:, 1] = allout[core, rows, cols0 + 1]
    return out_full
